# revision 2
# baseline (speedup 1.0000x reference)
"""AGNNConv (single-head) Trainium2 kernel v3, 8-core SPMD.

Reference:
    Xp  = X @ W                                   [N, 64]
    ef  = a * sum_d Xp[row]*Xp[col]               [E]   (SDDMM)
    out = segment_sum(ef[:,None] * Xp[col], row)        (SpMM)

Design (per core, dst-node sharded, edge-major, all-contiguous compute):
  * Phase 1: Xp rows written to DRAM as [feat|feat] duplicated bf16 256B rows,
    natural node order (contiguous writes).  Row space: [0:25088) = nodes
    0..25087 (pass A), [25088:25216) zeros, [25216:50304) = nodes 25088..50175
    (pass B, int16-reindexed), [50304:50432) zeros.  Pad gathers hit zeros.
  * Two passes by col range (int16 idx limit).  Per (core, pass) dsts are
    ranked by that pass's degree desc (order statistics align across cores, so
    the cross-core max padding is tight); 49 blocks of 128 ranks; greedy
    groups of blocks share a uniform slot count s.
  * Gather: non-transpose DRAM dma_gather, 256B elements, ~3.5k idx per
    instruction round-robin over 4 SWDGE queues (measured ~1.3-1.7ns/idx
    aggregate; concurrent queues are value-safe for non-transpose).  Stream
    slot-major: G[p, cell, :] with cell=(block, slot); slot 0 = the dst's own
    row (this pass's half, zero otherwise), and a small per-pass "od" gather
    holds the other half's dst rows.
  * Compute per group, all DVE, all contiguous/measured patterns:
      xpd = G[:,:,0:1,:] + od             (dst rows, two halves)
      t   = G[:,:,1:,:] * bcast(xpd)      (0.13 ns/elem)
      ef  = reduce_128(t)   -> [P,nb,s,1] (0.58 ns/elem; x2 from row dup)
      msg = G[:,:,1:,:] * bcast(ef)       (0.46 ns/elem)
      slot-halving tree (contiguous adds) -> block sums [P, nb, 128]
    a/2 is folded into one tiny tensor_scalar on the block sums.
  * Output: obst [128, 2, 49*64] f32 -> one DMA; host scales nothing (a/2
    folded on device), adds nothing: host inverse-permutes rank->node per
    pass and sums the two passes.  No scatter, no on-device transpose.
"""
import numpy as np
import ml_dtypes

import concourse.bass as bass
import concourse.tile as tile
from concourse import bacc, mybir
from concourse.bass_utils import run_bass_kernel_spmd

N = 50000
E = 800000
D = 64
NCORES = 8
NPC = N // NCORES            # 6250
HALF = 25088                 # pass A node range; pass B = [25088, 50176)
XROWS = 50432                # XP DRAM rows (with two 128-row zero fences)
ZPADA = 25100                # pass-A pad idx -> zero fence row
ZPADB = 25100                # pass-B pad idx (row 25216+25100=50316 -> zero)
NDST = 6272                  # ranks per pass (49 blocks)
NBLK = 49
CELLCAP = 96                 # max cells (rows of 256B) per group tile
CG = 28                      # max cells per gather instruction (3584 idx)
KQ = 4
KREPS = 1
KSTAGE = 4                   # 1=phase1, 2=+gathers, 3=+sddmm, 4=full

F32 = mybir.dt.float32
BF16 = mybir.dt.bfloat16
I16 = mybir.dt.int16
mult = mybir.AluOpType.mult
add = mybir.AluOpType.add
AX = mybir.AxisListType.X


def _wrap16(idx):
    idx = np.asarray(idx, np.int16)
    n = idx.size
    assert n % 16 == 0
    t = idx.reshape(n // 16, 16).T.copy()
    return np.tile(t, (8, 1))


def _prep(row, col):
    """Per-pass rank spaces, block slot counts, groups, per-core idx data."""
    row = np.asarray(row, np.int64)
    col = np.asarray(col, np.int64)
    cores = []
    for c in range(NCORES):
        m = (row // NPC) == c
        r = row[m] - c * NPC
        cl = col[m]
        ranks = np.full((2, NDST), -1, np.int64)
        degs = np.zeros((2, NDST), np.int64)
        for pa in range(2):
            dst = np.bincount(r[(cl >= HALF) == (pa == 1)], minlength=NPC)
            order = np.argsort(-dst, kind="stable")
            ranks[pa, :NPC] = order
            degs[pa, :NPC] = dst[order]
        cores.append(dict(r=r, cl=cl, ranks=ranks, degs=degs))

    # block slot counts (max over cores), then greedy groups of blocks
    sched = []           # per pass: list of (b0, nb, s)
    for pa in range(2):
        smax = np.max([cores[c]["degs"][pa] for c in range(NCORES)], axis=0)
        sblk = [max(1, int(smax[b * 128:(b + 1) * 128].max()))
                for b in range(NBLK)]
        groups = []
        i = 0
        while i < NBLK:
            m = sblk[i]
            j = i + 1
            while j < NBLK:
                nm = max(m, sblk[j])
                # group must fit the cell cap and not overpad
                if (j - i + 1) * (1 + nm) > CELLCAP:
                    break
                if (j - i + 1) * nm > sum(sblk[i:j + 1]) * 1.08 + 4:
                    break
                m = nm
                j += 1
            groups.append((i, j - i, m))
            i = j
        sched.append(groups)

    # per-core gather idx streams
    in_cores = []
    for c in range(NCORES):
        cr = cores[c]
        r, cl, ranksm, degs = cr["r"], cr["cl"], cr["ranks"], cr["degs"]
        gidx_parts, oidx_parts = [], []
        for pa in range(2):
            sel = (cl >= HALF) == (pa == 1)
            rs, ts = r[sel], cl[sel] - pa * HALF
            o = np.argsort(rs, kind="stable")
            rs, ts = rs[o], ts[o]
            starts = np.zeros(NPC + 1, np.int64)
            np.cumsum(np.bincount(rs, minlength=NPC), out=starts[1:])
            zpad = ZPADA
            # od stream: dst rows in the *other* pass's row space
            od = np.full((NBLK, 128), zpad, np.int64)
            for (b0, nb, s) in sched[pa]:
                cells = np.full((nb, 1 + s, 128), zpad, np.int64)
                for bl in range(nb):
                    for p in range(128):
                        rank = (b0 + bl) * 128 + p
                        node = ranksm[pa, rank]
                        if node < 0:
                            continue
                        gd = node + c * NPC
                        if (gd >= HALF) == (pa == 1):
                            cells[bl, 0, p] = gd - pa * HALF
                        else:
                            od[b0 + bl, p] = gd - (1 - pa) * HALF
                        d = int(degs[pa, rank])
                        e0 = int(starts[node])
                        cells[bl, 1:1 + d, p] = ts[e0:e0 + d]
                gidx_parts.append(cells.reshape(-1))
            oidx_parts.append(od.reshape(-1))
        in_cores.append(dict(gidx=np.concatenate(gidx_parts),
                             oidx=np.concatenate(oidx_parts)))
    ranks_all = [cores[c]["ranks"] for c in range(NCORES)]
    return sched, in_cores, ranks_all


def _build(a_val, sched, l_all):
    nc = bacc.Bacc("TRN2", target_bir_lowering=False, num_devices=NCORES,
                   num_swdge_queues=KQ)
    XT = nc.declare_dram_parameter("xt", [D, 50176], BF16, isOutput=False)
    Wp = nc.declare_dram_parameter("w", [D, D], BF16, isOutput=False)
    GIP = nc.declare_dram_parameter("gidx", [128, l_all // 16], I16,
                                    isOutput=False)
    OIP = nc.declare_dram_parameter("oidx", [128, 2 * NBLK * 128 // 16], I16,
                                    isOutput=False)
    OUT = nc.declare_dram_parameter("out", [128, 2 * NBLK * D], F32,
                                    isOutput=True)
    XP = nc.dram_tensor("xp", [XROWS, 128], BF16)

    with tile.TileContext(nc) as tc:
        with (
            tc.tile_pool(name="const", bufs=1) as cpool,
            tc.tile_pool(name="keep", bufs=1) as kpool,
        ):
            w_sb = cpool.tile([D, D], BF16)
            nc.sync.dma_start(w_sb[:], Wp[:])
            gidx_sb = cpool.tile([128, l_all // 16], I16, name="gidx_sb")
            nc.sync.dma_start(gidx_sb[:], GIP[:])
            oidx_sb = cpool.tile([128, 2 * NBLK * 128 // 16], I16,
                                 name="oidx_sb")
            nc.sync.dma_start(oidx_sb[:], OIP[:])
            zt = cpool.tile([128, 256], BF16, name="zt")
            nc.vector.memset(zt[:], 0.0)
            nc.sync.dma_start(
                XP[HALF:HALF + 128, :].rearrange("(a p) d -> p (a d)", p=128),
                zt[:, 0:128])
            nc.sync.dma_start(
                XP[XROWS - 128:XROWS, :].rearrange("(a p) d -> p (a d)", p=128),
                zt[:, 0:128])
            obst = kpool.tile([128, 2, NBLK, D], F32, name="obst")
            if KSTAGE < 4:
                nc.vector.memset(obst[:], 0.0)

            for _rep in range(KREPS):
                # ---- phase 1: Xp -> DRAM duplicated rows ----
                XB = 16   # node chunks (128 nodes) per xt tile
                with (
                    tc.tile_pool(name="xt", bufs=2) as xtpool,
                    tc.tile_pool(name="xps", bufs=2) as xpspool,
                    tc.tile_pool(name="ps1", bufs=2, space="PSUM") as ps1pool,
                ):
                    NCH = 50176 // 128      # 392 chunks
                    for g0 in range(0, NCH, XB):
                        nb = min(XB, NCH - g0)
                        xt_t = xtpool.tile([D, XB * 128], BF16)
                        nc.sync.dma_start(xt_t[:, 0:nb * 128],
                                          XT[:, g0 * 128:(g0 + nb) * 128])
                        xp_t = xpspool.tile([128, XB, 128], BF16)
                        for r4 in range(0, nb, 4):
                            n4 = min(4, nb - r4)
                            ps = ps1pool.tile([128, 256], F32)
                            for rr in range(n4):
                                nc.tensor.matmul(
                                    ps[:, rr * 64:rr * 64 + 64],
                                    lhsT=xt_t[:, (r4 + rr) * 128:(r4 + rr + 1) * 128],
                                    rhs=w_sb[:], start=True, stop=True)
                            nc.vector.tensor_copy(
                                out=xp_t[:, r4:r4 + n4, 0:64],
                                in_=ps[:, 0:n4 * 64].rearrange(
                                    "p (a d) -> p a d", d=64))
                            nc.scalar.copy(
                                out=xp_t[:, r4:r4 + n4, 64:128],
                                in_=ps[:, 0:n4 * 64].rearrange(
                                    "p (a d) -> p a d", d=64))
                        # rows: chunk k -> XP[k*128 (+128 if k >= 196)]
                        ro = g0 * 128 + (128 if g0 >= 196 else 0)
                        if g0 < 196 and g0 + nb > 196:
                            n1 = 196 - g0
                            nc.sync.dma_start(
                                XP[ro:ro + n1 * 128, :].rearrange(
                                    "(a p) d -> p a d", p=128),
                                xp_t[:, 0:n1, :])
                            nc.sync.dma_start(
                                XP[25216:25216 + (nb - n1) * 128, :].rearrange(
                                    "(a p) d -> p a d", p=128),
                                xp_t[:, n1:nb, :])
                        else:
                            nc.sync.dma_start(
                                XP[ro:ro + nb * 128, :].rearrange(
                                    "(a p) d -> p a d", p=128),
                                xp_t[:, 0:nb, :])

                # ---- phase 2 ----
                PASSES = 2 if KSTAGE >= 2 else 0
                qctr = 0
                with (
                    tc.tile_pool(name="g", bufs=2) as gpool,
                    tc.tile_pool(name="tm", bufs=2) as tmpool,
                    tc.tile_pool(name="sm", bufs=2) as smpool,
                    tc.tile_pool(name="tr", bufs=2) as trpool,
                ):
                    goff = 0
                    for pa in range(PASSES):
                        src = XP[pa * 25216:pa * 25216 + 25216, :]
                        osrc = XP[(1 - pa) * 25216:(1 - pa) * 25216 + 25216, :]
                        od_t = smpool.tile([128, NBLK, 128], BF16, name="od")
                        nc.gpsimd.dma_gather(
                            out_ap=od_t[:], in_ap=osrc,
                            idxs_ap=oidx_sb[:, pa * NBLK * 8:(pa + 1) * NBLK * 8],
                            num_idxs=NBLK * 128, num_idxs_reg=NBLK * 128,
                            elem_size=128, single_packet=False,
                            queue_num=qctr % KQ)
                        qctr += 1
                        for (b0, nbg, s) in sched[pa]:
                            cells = nbg * (1 + s)
                            g = gpool.tile([128, CELLCAP, 128], BF16, name="g")
                            co = 0
                            while co < cells:
                                cl = min(CG, cells - co)
                                nc.gpsimd.dma_gather(
                                    out_ap=g[:, co:co + cl, :],
                                    in_ap=src,
                                    idxs_ap=gidx_sb[:, goff // 16:
                                                    (goff + cl * 128) // 16],
                                    num_idxs=cl * 128, num_idxs_reg=cl * 128,
                                    elem_size=128, single_packet=False,
                                    queue_num=qctr % KQ)
                                qctr += 1
                                goff += cl * 128
                                co += cl
                            if KSTAGE < 3:
                                continue
                            gv = g[:, 0:cells, :].rearrange(
                                "p (b s) d -> p b s d", b=nbg)
                            xpd = smpool.tile([128, nbg, 1, 128], BF16,
                                              name="xpd")
                            nc.vector.tensor_tensor(
                                out=xpd[:], in0=gv[:, :, 0:1, :],
                                in1=od_t[:, b0:b0 + nbg, :].rearrange(
                                    "p b (o d) -> p b o d", o=1),
                                op=add)
                            t_t = tmpool.tile([128, nbg, s, 128], BF16,
                                              name="t")
                            nc.vector.tensor_tensor(
                                out=t_t[:], in0=gv[:, :, 1:1 + s, :],
                                in1=xpd[:].to_broadcast([128, nbg, s, 128]),
                                op=mult)
                            ef = smpool.tile([128, nbg, s, 1], F32, name="ef")
                            nc.vector.tensor_reduce(
                                out=ef[:], in_=t_t[:], axis=AX, op=add)
                            if KSTAGE < 4:
                                continue
                            msg = t_t   # overwrite t
                            nc.vector.tensor_tensor(
                                out=msg[:], in0=gv[:, :, 1:1 + s, :],
                                in1=ef[:].to_broadcast([128, nbg, s, 128]),
                                op=mult)
                            # slot-halving tree down to 1 slot
                            red = msg[:]
                            cur = s
                            lvl = 0
                            while cur > 1:
                                h = cur // 2
                                odd = cur - 2 * h
                                dst_t = trpool.tile([128, nbg, h + odd, 128],
                                                    BF16, name=f"tr{lvl % 2}")
                                nc.vector.tensor_tensor(
                                    out=dst_t[:, :, 0:h, :],
                                    in0=red[:, :, 0:h, :],
                                    in1=red[:, :, h:2 * h, :], op=add)
                                if odd:
                                    nc.vector.tensor_copy(
                                        out=dst_t[:, :, h:h + 1, :],
                                        in_=red[:, :, 2 * h:cur, :])
                                red = dst_t[:]
                                cur = h + odd
                                lvl += 1
                            nc.vector.tensor_scalar_mul(
                                obst[:, pa, b0:b0 + nbg, :],
                                red[:, :, 0, 0:64], float(a_val) * 0.5)
                nc.sync.dma_start(
                    OUT[:], obst[:].rearrange("p a b d -> p (a b d)"))
    nc.compile()
    return nc


def _make_inputs(X, weights, row, col):
    sched, in_cores, ranks_all = _prep(row, col)
    l_all = sum(128 * nb * (1 + s) for pa in range(2)
                for (_, nb, s) in sched[pa])
    XTpad = np.zeros((D, 50176), np.float32)
    XTpad[:, :N] = np.asarray(X, np.float32).T
    xt_bf = XTpad.astype(ml_dtypes.bfloat16)
    w_bf = np.asarray(weights, np.float32).astype(ml_dtypes.bfloat16)
    in_maps = [
        dict(xt=xt_bf, w=w_bf,
             gidx=_wrap16(in_cores[c]["gidx"]),
             oidx=_wrap16(in_cores[c]["oidx"]))
        for c in range(NCORES)
    ]
    return sched, l_all, in_maps, ranks_all


def _postprocess(results, ranks_all):
    outs = []
    for c in range(NCORES):
        o = np.asarray(results[c]["out"], np.float32).reshape(
            128, 2, NBLK, D)
        nat = np.zeros((NPC, D), np.float32)
        for pa in range(2):
            vals = o[:, pa, :, :].transpose(1, 0, 2).reshape(NDST, D)
            ranksm = ranks_all[c][pa]
            valid = ranksm >= 0
            nat[ranksm[valid]] += vals[valid]
        outs.append(nat)
    return np.concatenate(outs, 0)[:N].astype(np.float32)


def kernel(X, weights, attention_w, row, col):
    X = np.ascontiguousarray(np.asarray(X, np.float32))
    weights = np.ascontiguousarray(np.asarray(weights, np.float32))
    a = float(np.asarray(attention_w).reshape(-1)[0])
    row = np.asarray(row, np.int64)
    col = np.asarray(col, np.int64)

    sched, l_all, in_maps, ranks_all = _make_inputs(X, weights, row, col)
    nc = _build(a, sched, l_all)
    res = run_bass_kernel_spmd(nc, in_maps, list(range(NCORES)))
    return _postprocess(res.results, ranks_all)


# revision 3
# speedup vs baseline: 1.0465x; 1.0465x over previous
"""AGNNConv (single-head) Trainium2 kernel v3, 8-core SPMD.

Reference:
    Xp  = X @ W                                   [N, 64]
    ef  = a * sum_d Xp[row]*Xp[col]               [E]   (SDDMM)
    out = segment_sum(ef[:,None] * Xp[col], row)        (SpMM)

Design (per core, dst-node sharded, edge-major, all-contiguous compute):
  * Phase 1: Xp rows written to DRAM as [feat|feat] duplicated bf16 256B rows,
    natural node order (contiguous writes).  Row space: [0:25088) = nodes
    0..25087 (pass A), [25088:25216) zeros, [25216:50304) = nodes 25088..50175
    (pass B, int16-reindexed), [50304:50432) zeros.  Pad gathers hit zeros.
  * Two passes by col range (int16 idx limit).  Per (core, pass) dsts are
    ranked by that pass's degree desc (order statistics align across cores, so
    the cross-core max padding is tight); 49 blocks of 128 ranks; greedy
    groups of blocks share a uniform slot count s.
  * Gather: non-transpose DRAM dma_gather, 256B elements, ~3.5k idx per
    instruction round-robin over 4 SWDGE queues (measured ~1.3-1.7ns/idx
    aggregate; concurrent queues are value-safe for non-transpose).  Stream
    slot-major: G[p, cell, :] with cell=(block, slot); slot 0 = the dst's own
    row (this pass's half, zero otherwise), and a small per-pass "od" gather
    holds the other half's dst rows.
  * Compute per group, all DVE, all contiguous/measured patterns:
      xpd = G[:,:,0:1,:] + od             (dst rows, two halves)
      t   = G[:,:,1:,:] * bcast(xpd)      (0.13 ns/elem)
      ef  = reduce_128(t)   -> [P,nb,s,1] (0.58 ns/elem; x2 from row dup)
      msg = G[:,:,1:,:] * bcast(ef)       (0.46 ns/elem)
      slot-halving tree (contiguous adds) -> block sums [P, nb, 128]
    a/2 is folded into one tiny tensor_scalar on the block sums.
  * Output: obst [128, 2, 49*64] f32 -> one DMA; host scales nothing (a/2
    folded on device), adds nothing: host inverse-permutes rank->node per
    pass and sums the two passes.  No scatter, no on-device transpose.
"""
import numpy as np
import ml_dtypes

import concourse.bass as bass
import concourse.tile as tile
from concourse import bacc, mybir
from concourse.bass_utils import run_bass_kernel_spmd

N = 50000
E = 800000
D = 64
NCORES = 8
NPC = N // NCORES            # 6250
HALF = 25088                 # pass A node range; pass B = [25088, 50176)
XROWS = 50432                # XP DRAM rows (with two 128-row zero fences)
ZPADA = 25100                # pass-A pad idx -> zero fence row
ZPADB = 25100                # pass-B pad idx (row 25216+25100=50316 -> zero)
NDST = 6272                  # ranks per pass (49 blocks)
NBLK = 49
CELLCAP = 72                 # max cells (rows of 256B) per group tile
CG = 28                      # max cells per gather instruction (3584 idx)
KQ = 4
KREPS = 1
KSTAGE = 4                   # 1=phase1, 2=+gathers, 3=+sddmm, 4=full

F32 = mybir.dt.float32
BF16 = mybir.dt.bfloat16
I16 = mybir.dt.int16
mult = mybir.AluOpType.mult
add = mybir.AluOpType.add
AX = mybir.AxisListType.X


def _wrap16(idx):
    idx = np.asarray(idx, np.int16)
    n = idx.size
    assert n % 16 == 0
    t = idx.reshape(n // 16, 16).T.copy()
    return np.tile(t, (8, 1))


def _prep(row, col):
    """Per-pass rank spaces, block slot counts, groups, per-core idx data."""
    row = np.asarray(row, np.int64)
    col = np.asarray(col, np.int64)
    cores = []
    for c in range(NCORES):
        m = (row // NPC) == c
        r = row[m] - c * NPC
        cl = col[m]
        ranks = np.full((2, NDST), -1, np.int64)
        degs = np.zeros((2, NDST), np.int64)
        for pa in range(2):
            dst = np.bincount(r[(cl >= HALF) == (pa == 1)], minlength=NPC)
            order = np.argsort(-dst, kind="stable")
            ranks[pa, :NPC] = order
            degs[pa, :NPC] = dst[order]
        cores.append(dict(r=r, cl=cl, ranks=ranks, degs=degs))

    # block slot counts (max over cores), then greedy groups of blocks
    sched = []           # per pass: list of (b0, nb, s)
    for pa in range(2):
        smax = np.max([cores[c]["degs"][pa] for c in range(NCORES)], axis=0)
        sblk = [max(1, int(smax[b * 128:(b + 1) * 128].max()))
                for b in range(NBLK)]
        groups = []
        i = 0
        while i < NBLK:
            m = sblk[i]
            j = i + 1
            while j < NBLK:
                nm = max(m, sblk[j])
                # group must fit the cell cap and not overpad
                if (j - i + 1) * (1 + nm) > CELLCAP:
                    break
                if (j - i + 1) * nm > sum(sblk[i:j + 1]) * 1.08 + 4:
                    break
                m = nm
                j += 1
            groups.append((i, j - i, m))
            i = j
        sched.append(groups)

    # per-core gather idx streams
    in_cores = []
    for c in range(NCORES):
        cr = cores[c]
        r, cl, ranksm, degs = cr["r"], cr["cl"], cr["ranks"], cr["degs"]
        gidx_parts, oidx_parts = [], []
        for pa in range(2):
            sel = (cl >= HALF) == (pa == 1)
            rs, ts = r[sel], cl[sel] - pa * HALF
            o = np.argsort(rs, kind="stable")
            rs, ts = rs[o], ts[o]
            starts = np.zeros(NPC + 1, np.int64)
            np.cumsum(np.bincount(rs, minlength=NPC), out=starts[1:])
            zpad = ZPADA
            # od stream: dst rows in the *other* pass's row space
            od = np.full((NBLK, 128), zpad, np.int64)
            for (b0, nb, s) in sched[pa]:
                cells = np.full((nb, 1 + s, 128), zpad, np.int64)
                for bl in range(nb):
                    for p in range(128):
                        rank = (b0 + bl) * 128 + p
                        node = ranksm[pa, rank]
                        if node < 0:
                            continue
                        gd = node + c * NPC
                        if (gd >= HALF) == (pa == 1):
                            cells[bl, 0, p] = gd - pa * HALF
                        else:
                            od[b0 + bl, p] = gd - (1 - pa) * HALF
                        d = int(degs[pa, rank])
                        e0 = int(starts[node])
                        cells[bl, 1:1 + d, p] = ts[e0:e0 + d]
                gidx_parts.append(cells.reshape(-1))
            oidx_parts.append(od.reshape(-1))
        in_cores.append(dict(gidx=np.concatenate(gidx_parts),
                             oidx=np.concatenate(oidx_parts)))
    ranks_all = [cores[c]["ranks"] for c in range(NCORES)]
    return sched, in_cores, ranks_all


def _build(a_val, sched, l_all, gbufs=3):
    nc = bacc.Bacc("TRN2", target_bir_lowering=False, num_devices=NCORES,
                   num_swdge_queues=KQ)
    XT = nc.declare_dram_parameter("xt", [D, 50176], BF16, isOutput=False)
    Wp = nc.declare_dram_parameter("w", [D, D], BF16, isOutput=False)
    GIP = nc.declare_dram_parameter("gidx", [128, l_all // 16], I16,
                                    isOutput=False)
    OIP = nc.declare_dram_parameter("oidx", [128, 2 * NBLK * 128 // 16], I16,
                                    isOutput=False)
    OUT = nc.declare_dram_parameter("out", [128, 2 * NBLK * D], F32,
                                    isOutput=True)
    XPA = nc.dram_tensor("xpa", [25216, 128], BF16)
    XPB = nc.dram_tensor("xpb", [25216, 128], BF16)

    with tile.TileContext(nc) as tc:
        with (
            tc.tile_pool(name="const", bufs=1) as cpool,
            tc.tile_pool(name="keep", bufs=1) as kpool,
        ):
            w_sb = cpool.tile([D, D], BF16)
            nc.sync.dma_start(w_sb[:], Wp[:])
            gidx_sb = cpool.tile([128, l_all // 16], I16, name="gidx_sb")
            nc.sync.dma_start(gidx_sb[:], GIP[:])
            oidx_sb = cpool.tile([128, 2 * NBLK * 128 // 16], I16,
                                 name="oidx_sb")
            nc.sync.dma_start(oidx_sb[:], OIP[:])
            zt = cpool.tile([128, 256], BF16, name="zt")
            nc.vector.memset(zt[:], 0.0)
            nc.sync.dma_start(
                XPA[HALF:HALF + 128, :].rearrange("(a p) d -> p (a d)", p=128),
                zt[:, 0:128])
            nc.sync.dma_start(
                XPB[25088:25216, :].rearrange("(a p) d -> p (a d)", p=128),
                zt[:, 0:128])
            obst = kpool.tile([128, 2, NBLK, D], F32, name="obst")
            if KSTAGE < 4:
                nc.vector.memset(obst[:], 0.0)

            for _rep in range(KREPS):
                # ---- phase 1: Xp -> DRAM duplicated rows ----
                XB = 16   # node chunks (128 nodes) per xt tile
                with (
                    tc.tile_pool(name="xt", bufs=2) as xtpool,
                    tc.tile_pool(name="xps", bufs=2) as xpspool,
                    tc.tile_pool(name="ps1", bufs=2, space="PSUM") as ps1pool,
                ):
                    NCH = 50176 // 128      # 392 chunks
                    for g0 in range(0, NCH, XB):
                        nb = min(XB, NCH - g0)
                        xt_t = xtpool.tile([D, XB * 128], BF16)
                        nc.sync.dma_start(xt_t[:, 0:nb * 128],
                                          XT[:, g0 * 128:(g0 + nb) * 128])
                        xp_t = xpspool.tile([128, XB, 128], BF16)
                        for r4 in range(0, nb, 4):
                            n4 = min(4, nb - r4)
                            ps = ps1pool.tile([128, 256], F32)
                            for rr in range(n4):
                                nc.tensor.matmul(
                                    ps[:, rr * 64:rr * 64 + 64],
                                    lhsT=xt_t[:, (r4 + rr) * 128:(r4 + rr + 1) * 128],
                                    rhs=w_sb[:], start=True, stop=True)
                            nc.vector.tensor_copy(
                                out=xp_t[:, r4:r4 + n4, 0:64],
                                in_=ps[:, 0:n4 * 64].rearrange(
                                    "p (a d) -> p a d", d=64))
                            nc.scalar.copy(
                                out=xp_t[:, r4:r4 + n4, 64:128],
                                in_=ps[:, 0:n4 * 64].rearrange(
                                    "p (a d) -> p a d", d=64))
                        # chunk k -> XPA rows k*128, or XPB rows (k-196)*128
                        if g0 < 196 and g0 + nb > 196:
                            n1 = 196 - g0
                            nc.sync.dma_start(
                                XPA[g0 * 128:(g0 + n1) * 128, :].rearrange(
                                    "(a p) d -> p a d", p=128),
                                xp_t[:, 0:n1, :])
                            nc.sync.dma_start(
                                XPB[0:(nb - n1) * 128, :].rearrange(
                                    "(a p) d -> p a d", p=128),
                                xp_t[:, n1:nb, :])
                        elif g0 < 196:
                            nc.sync.dma_start(
                                XPA[g0 * 128:(g0 + nb) * 128, :].rearrange(
                                    "(a p) d -> p a d", p=128),
                                xp_t[:, 0:nb, :])
                        else:
                            ro = (g0 - 196) * 128
                            nc.sync.dma_start(
                                XPB[ro:ro + nb * 128, :].rearrange(
                                    "(a p) d -> p a d", p=128),
                                xp_t[:, 0:nb, :])

                # ---- phase 2 ----
                PASSES = 2 if KSTAGE >= 2 else 0
                qctr = 0
                with (
                    tc.tile_pool(name="g", bufs=gbufs) as gpool,
                    tc.tile_pool(name="tm", bufs=gbufs) as tmpool,
                    tc.tile_pool(name="sm", bufs=2) as smpool,
                    tc.tile_pool(name="tr", bufs=2) as trpool,
                ):
                    goff = 0
                    for pa in range(PASSES):
                        src = (XPA if pa == 0 else XPB)[:]
                        osrc = (XPB if pa == 0 else XPA)[:]
                        od_t = smpool.tile([128, NBLK, 128], BF16, name="od")
                        nc.gpsimd.dma_gather(
                            out_ap=od_t[:], in_ap=osrc,
                            idxs_ap=oidx_sb[:, pa * NBLK * 8:(pa + 1) * NBLK * 8],
                            num_idxs=NBLK * 128, num_idxs_reg=NBLK * 128,
                            elem_size=128, single_packet=False,
                            queue_num=qctr % KQ)
                        qctr += 1
                        for (b0, nbg, s) in sched[pa]:
                            cells = nbg * (1 + s)
                            g = gpool.tile([128, CELLCAP, 128], BF16, name="g")
                            co = 0
                            while co < cells:
                                cl = min(CG, cells - co)
                                nc.gpsimd.dma_gather(
                                    out_ap=g[:, co:co + cl, :],
                                    in_ap=src,
                                    idxs_ap=gidx_sb[:, goff // 16:
                                                    (goff + cl * 128) // 16],
                                    num_idxs=cl * 128, num_idxs_reg=cl * 128,
                                    elem_size=128, single_packet=False,
                                    queue_num=qctr % KQ)
                                qctr += 1
                                goff += cl * 128
                                co += cl
                            if KSTAGE < 3:
                                continue
                            gv = g[:, 0:cells, :].rearrange(
                                "p (b s) d -> p b s d", b=nbg)
                            xpd = smpool.tile([128, nbg, 1, 128], BF16,
                                              name="xpd")
                            nc.vector.tensor_tensor(
                                out=xpd[:], in0=gv[:, :, 0:1, :],
                                in1=od_t[:, b0:b0 + nbg, :].rearrange(
                                    "p b (o d) -> p b o d", o=1),
                                op=add)
                            t_t = tmpool.tile([128, nbg, s, 128], BF16,
                                              name="t")
                            nc.vector.tensor_tensor(
                                out=t_t[:], in0=gv[:, :, 1:1 + s, :],
                                in1=xpd[:].to_broadcast([128, nbg, s, 128]),
                                op=mult)
                            ef = smpool.tile([128, nbg, s, 1], F32, name="ef")
                            nc.vector.tensor_reduce(
                                out=ef[:], in_=t_t[:], axis=AX, op=add)
                            if KSTAGE < 4:
                                continue
                            msg = t_t   # overwrite t
                            nc.vector.tensor_tensor(
                                out=msg[:], in0=gv[:, :, 1:1 + s, :],
                                in1=ef[:].to_broadcast([128, nbg, s, 128]),
                                op=mult)
                            # slot-halving tree down to 1 slot
                            red = msg[:]
                            cur = s
                            lvl = 0
                            while cur > 1:
                                h = cur // 2
                                odd = cur - 2 * h
                                dst_t = trpool.tile([128, nbg, h + odd, 128],
                                                    BF16, name=f"tr{lvl % 2}")
                                nc.vector.tensor_tensor(
                                    out=dst_t[:, :, 0:h, :],
                                    in0=red[:, :, 0:h, :],
                                    in1=red[:, :, h:2 * h, :], op=add)
                                if odd:
                                    nc.vector.tensor_copy(
                                        out=dst_t[:, :, h:h + 1, :],
                                        in_=red[:, :, 2 * h:cur, :])
                                red = dst_t[:]
                                cur = h + odd
                                lvl += 1
                            nc.vector.tensor_scalar_mul(
                                obst[:, pa, b0:b0 + nbg, :],
                                red[:, :, 0, 0:64], float(a_val) * 0.5)
                nc.sync.dma_start(
                    OUT[:], obst[:].rearrange("p a b d -> p (a b d)"))
    nc.compile()
    return nc


def _make_inputs(X, weights, row, col):
    sched, in_cores, ranks_all = _prep(row, col)
    l_all = sum(128 * nb * (1 + s) for pa in range(2)
                for (_, nb, s) in sched[pa])
    XTpad = np.zeros((D, 50176), np.float32)
    XTpad[:, :N] = np.asarray(X, np.float32).T
    xt_bf = XTpad.astype(ml_dtypes.bfloat16)
    w_bf = np.asarray(weights, np.float32).astype(ml_dtypes.bfloat16)
    in_maps = [
        dict(xt=xt_bf, w=w_bf,
             gidx=_wrap16(in_cores[c]["gidx"]),
             oidx=_wrap16(in_cores[c]["oidx"]))
        for c in range(NCORES)
    ]
    return sched, l_all, in_maps, ranks_all


def _postprocess(results, ranks_all):
    outs = []
    for c in range(NCORES):
        o = np.asarray(results[c]["out"], np.float32).reshape(
            128, 2, NBLK, D)
        nat = np.zeros((NPC, D), np.float32)
        for pa in range(2):
            vals = o[:, pa, :, :].transpose(1, 0, 2).reshape(NDST, D)
            ranksm = ranks_all[c][pa]
            valid = ranksm >= 0
            nat[ranksm[valid]] += vals[valid]
        outs.append(nat)
    return np.concatenate(outs, 0)[:N].astype(np.float32)


def kernel(X, weights, attention_w, row, col):
    X = np.ascontiguousarray(np.asarray(X, np.float32))
    weights = np.ascontiguousarray(np.asarray(weights, np.float32))
    a = float(np.asarray(attention_w).reshape(-1)[0])
    row = np.asarray(row, np.int64)
    col = np.asarray(col, np.int64)

    sched, l_all, in_maps, ranks_all = _make_inputs(X, weights, row, col)
    nc = _build(a, sched, l_all)
    res = run_bass_kernel_spmd(nc, in_maps, list(range(NCORES)))
    return _postprocess(res.results, ranks_all)
